# revision 8
# baseline (speedup 1.0000x reference)
"""MoE gate kernel (nn_Gate) for Trainium2, 8 NeuronCores, data-parallel over tokens.

Per core (8192 tokens):
  - logits.T = W @ x_shard.T on PE with the [2048,64] weight as the stationary
    operand (fp32, K accumulated in PSUM over 16 chunks, N=512 tokens/matmul).
  - PE-transpose of the [64, 512] logit groups back to [128 tokens, 64].
  - softmax (ACT exp with fused row-sum) + group-limited top-6 via the DVE
    top-8 sort unit (max / max_index).
x is pre-tiled on the host so each 4 MiB group DMA is 32 KiB-contiguous per
partition; W is transposed on the host.

Outputs per core: packed [tokens, 16] rows (top-8 f32 weights + top-8 indices
as u32 bits) and a [128, 64] partial sum of softmax scores. The tiny [64]
expert statistics (counts / mean scores) are reduced on the host, where
moe_loss is computed exactly as the reference does.
"""
import functools
from concurrent.futures import ThreadPoolExecutor
from contextlib import ExitStack

import numpy as np

import concourse.bass as bass
import concourse.mybir as mybir
import concourse.tile as tile
from concourse import bacc
from concourse.bass_utils import run_bass_kernel_spmd

TOKENS = 65536
DIM = 2048
E = 64
NG = 8
GS = E // NG
TOPK = 6
ROUTE_SCALE = 1.0
BAL = 1e-05
NCORES = 8
TPC = TOKENS // NCORES          # tokens per core
GT = 512                        # tokens per matmul group
NGRP = TPC // GT                # matmul groups per core
TILE_T = 128                    # tokens per selection tile
NSUB = GT // TILE_T             # selection tiles per group
KC = DIM // 128                 # contraction chunks

F32 = mybir.dt.float32
NEG_BIG = -1e30


def _emit_select(nc, pools, logits_ps, pack, tok0, acc):
    """Softmax + group-limited top-6 for one [128, 64] logits tile (PSUM)."""
    work, opool = pools
    nmax = work.tile([128, 1], F32)
    nc.vector.tensor_reduce(out=nmax, in_=logits_ps, axis=mybir.AxisListType.X,
                            op=mybir.AluOpType.max, negate=True)
    e_t = work.tile([128, E], F32)
    esum = work.tile([128, 1], F32)
    nc.scalar.activation(out=e_t, in_=logits_ps,
                         func=mybir.ActivationFunctionType.Exp,
                         bias=nmax, scale=1.0, accum_out=esum)
    recip = work.tile([128, 1], F32)
    nc.vector.reciprocal(out=recip, in_=esum)
    scores = work.tile([128, E], F32)
    nc.scalar.activation(out=scores, in_=e_t,
                         func=mybir.ActivationFunctionType.Copy, scale=recip)
    nc.gpsimd.tensor_tensor(out=acc, in0=acc, in1=scores, op=mybir.AluOpType.add)

    # group scores: top-2 sum within each group of 8
    s3 = scores.rearrange("p (g d) -> p g d", g=NG)
    g1 = work.tile([128, NG], F32)
    nc.vector.tensor_reduce(out=g1, in_=s3, axis=mybir.AxisListType.X,
                            op=mybir.AluOpType.max)
    eqb = work.tile([128, NG, GS], F32)
    nc.vector.tensor_tensor(out=eqb, in0=s3,
                            in1=g1[:, :, None].to_broadcast([128, NG, GS]),
                            op=mybir.AluOpType.is_equal)
    x2 = work.tile([128, NG, GS], F32)
    nc.vector.scalar_tensor_tensor(out=x2, in0=eqb, scalar=NEG_BIG, in1=s3,
                                   op0=mybir.AluOpType.mult,
                                   op1=mybir.AluOpType.add)
    g2 = work.tile([128, NG], F32)
    nc.vector.tensor_reduce(out=g2, in_=x2, axis=mybir.AxisListType.X,
                            op=mybir.AluOpType.max)
    gsum = work.tile([128, NG], F32)
    nc.vector.tensor_tensor(out=gsum, in0=g1, in1=g2, op=mybir.AluOpType.add)

    # keep top-4 groups: drop mask -> -BIG penalty on dropped groups
    gs8 = work.tile([128, 8], F32)
    nc.vector.max(out=gs8, in_=gsum)
    drop = work.tile([128, NG], F32)
    nc.vector.tensor_scalar(out=drop, in0=gsum, scalar1=gs8[:, 3:4],
                            scalar2=None, op0=mybir.AluOpType.is_lt)
    sm = work.tile([128, NG, GS], F32)
    nc.vector.scalar_tensor_tensor(
        out=sm, in0=drop[:, :, None].to_broadcast([128, NG, GS]),
        scalar=NEG_BIG, in1=s3, op0=mybir.AluOpType.mult, op1=mybir.AluOpType.add)

    # top-8 values + indices; first 6 are the routed experts
    stage = opool.tile([128, 16], F32)
    smf = sm.rearrange("p g d -> p (g d)")
    nc.vector.max(out=stage[:, 0:8], in_=smf)
    nc.vector.max_index(out=stage[:, 8:16].bitcast(mybir.dt.uint32),
                        in_max=stage[:, 0:8], in_values=smf)
    nc.gpsimd.dma_start(out=pack[tok0:tok0 + TILE_T, :], in_=stage)


def _emit_group(nc, g, xh, pack, pools, wt_sb, acc, ident):
    xpool, psuml, psumt, work, opool = pools
    xg = xpool.tile([128, KC, GT], F32)
    nc.sync.dma_start(out=xg, in_=xh[g * 128:(g + 1) * 128, :].rearrange(
        "p (c n) -> p c n", c=KC))
    ps_l = psuml.tile([64, GT], F32)
    for c in range(KC):
        nc.tensor.matmul(ps_l, lhsT=wt_sb[:, c, :], rhs=xg[:, c, :],
                         start=(c == 0), stop=(c == KC - 1))
    lsb = work.tile([64, GT], F32)
    nc.scalar.copy(out=lsb, in_=ps_l)
    for m in range(NSUB):
        pst = psumt.tile([128, E], F32)
        nc.tensor.transpose(pst, lsb[:, m * TILE_T:(m + 1) * TILE_T], ident)
        _emit_select(nc, (work, opool), pst, pack, g * GT + m * TILE_T, acc)


def _make_nc():
    nc = bacc.Bacc("TRN2", target_bir_lowering=False, debug=False,
                   num_devices=NCORES)
    xh = nc.dram_tensor("xh", [NGRP * 128, KC * GT], F32, kind="ExternalInput")
    wt = nc.dram_tensor("wt", [DIM, E], F32, kind="ExternalInput")
    pack = nc.dram_tensor("pack", [TPC, 16], F32, kind="ExternalOutput")
    sacc_out = nc.dram_tensor("sacc", [128, E], F32, kind="ExternalOutput")
    return nc, xh, wt, pack, sacc_out


def _emit_prelude(nc, tc, ctx):
    singles = ctx.enter_context(tc.tile_pool(name="singles", bufs=1))
    xpool = ctx.enter_context(tc.tile_pool(name="xpool", bufs=3))
    psuml = ctx.enter_context(tc.tile_pool(name="psuml", bufs=2, space="PSUM"))
    psumt = ctx.enter_context(tc.tile_pool(name="psumt", bufs=4, space="PSUM"))
    work = ctx.enter_context(tc.tile_pool(name="work", bufs=3))
    opool = ctx.enter_context(tc.tile_pool(name="opool", bufs=4))
    pools = (xpool, psuml, psumt, work, opool)

    acc = singles.tile([128, E], F32)
    nc.vector.memset(acc, 0.0)
    # identity for PE transpose
    ident = singles.tile([64, 64], F32)
    ones = singles.tile([64, 64], F32)
    nc.vector.memset(ones, 1.0)
    nc.gpsimd.affine_select(out=ident, in_=ones, pattern=[[1, 64]],
                            compare_op=mybir.AluOpType.is_equal, fill=0.0,
                            base=0, channel_multiplier=-1)
    return singles, pools, acc, ident


@functools.lru_cache(maxsize=4)
def _build_reps(reps):
    nc, xh, wt, pack, sacc_out = _make_nc()
    with tile.TileContext(nc) as tc, ExitStack() as ctx:
        singles, pools, acc, ident = _emit_prelude(nc, tc, ctx)
        wt_sb = singles.tile([128, KC, E], F32)
        nc.sync.dma_start(out=wt_sb,
                          in_=wt.ap().rearrange("(c p) e -> p c e", p=128))
        for _ in range(reps):
            for g in range(NGRP):
                _emit_group(nc, g, xh, pack, pools, wt_sb, acc, ident)
        nc.sync.dma_start(out=sacc_out[:, :], in_=acc)
    nc.compile()
    return nc


@functools.lru_cache(maxsize=1)
def _build_loop():
    """Timing variant: device-side dynamic loop; trip count from input nrep."""
    nc, xh, wt, pack, sacc_out = _make_nc()
    nrep = nc.dram_tensor("nrep", [1, 1], mybir.dt.uint32, kind="ExternalInput")
    with tile.TileContext(nc) as tc, ExitStack() as ctx:
        singles, pools, acc, ident = _emit_prelude(nc, tc, ctx)
        wt_sb = singles.tile([128, KC, E], F32)
        nc.sync.dma_start(out=wt_sb,
                          in_=wt.ap().rearrange("(c p) e -> p c e", p=128))
        nrep_sb = singles.tile([1, 1], mybir.dt.uint32)
        nc.sync.dma_start(out=nrep_sb, in_=nrep[:, :])
        k = nc.values_load(nrep_sb[0:1, 0:1], min_val=1, max_val=1 << 20,
                           skip_runtime_bounds_check=True)
        with tc.For_i(0, k, 1):
            for g in range(NGRP):
                _emit_group(nc, g, xh, pack, pools, wt_sb, acc, ident)
        nc.sync.dma_start(out=sacc_out[:, :], in_=acc)
    nc.compile()
    return nc


def _build():
    return _build_reps(1)


def _prep_shard(x, i):
    """[TPC, DIM] shard -> pre-tiled [NGRP*128, KC*GT] so each group's DMA
    reads 32 KiB contiguous per partition: xh[g*128+p, c*GT+n] = x[g*GT+n, c*128+p]."""
    sh = x[i * TPC:(i + 1) * TPC]
    return np.ascontiguousarray(
        sh.reshape(NGRP, GT, KC, 128).transpose(0, 3, 2, 1).reshape(
            NGRP * 128, KC * GT))


def kernel(x, weight, bias):
    x = np.asarray(x, dtype=np.float32)
    weight = np.asarray(weight, dtype=np.float32)
    nc = _build()
    wt = np.ascontiguousarray(weight.T)
    with ThreadPoolExecutor(NCORES) as ex:
        xhs = list(ex.map(lambda i: _prep_shard(x, i), range(NCORES)))
    in_maps = [{"xh": xhs[i], "wt": wt} for i in range(NCORES)]
    results = run_bass_kernel_spmd(nc, in_maps, core_ids=list(range(NCORES))).results

    packs = [r["pack"] for r in results]
    w = np.concatenate([p[:, 0:TOPK] for p in packs], axis=0).astype(np.float32)
    if ROUTE_SCALE != 1.0:
        w = (w * ROUTE_SCALE).astype(np.float32)
    idx = np.concatenate([p.view(np.int32)[:, 8:8 + TOPK] for p in packs], axis=0)

    # load-balance aux loss from tiny [E] statistics (host reduction)
    score_sum = np.zeros(E, np.float64)
    for r in results:
        score_sum += r["sacc"].astype(np.float64).sum(axis=0)
    counts = np.bincount(idx.ravel(), minlength=E).astype(np.float32)
    f = counts / np.float32(TOKENS) * np.float32(E) / np.float32(TOPK)
    p_vec = (score_sum / TOKENS).astype(np.float32)
    moe_loss = np.float32((p_vec * f).sum() * BAL)
    return w, idx, moe_loss
